# revision 4
# baseline (speedup 1.0000x reference)
import os
import hashlib
import numpy as np

B, S, D, H, DK = 2, 2048, 1024, 16, 64
NCORES = 8
GROUPS = 4
HPG = 4
GDIM = HPG * DK
NEG = -1.0e9
QB = 512
NQB = S // QB
NKT = S // 128
NDM = D // 128

MODE = os.environ.get("BASS_MHA_MODE", "bf16")

_CACHE = {}


def _make_plan(m2d):
    sub = np.asarray(m2d).reshape(S // 128, 128, S // 128, 128)
    any_ = sub.any(axis=(1, 3))
    all_ = sub.all(axis=(1, 3))

    uniq = {}
    uniq_src = []
    plan = []
    for qb in range(NQB):
        entries = []
        cs = list(range(4 * qb, 4 * qb + 4))
        for j in range(NKT):
            states = []
            for c in cs:
                if not any_[c, j]:
                    states.append("skip")
                elif all_[c, j]:
                    states.append("full")
                else:
                    states.append("mixed")
            if all(s == "skip" for s in states):
                continue
            cmin = next(i for i, s in enumerate(states) if s != "skip")
            mult_cols = []
            for i in range(cmin, 4):
                if states[i] == "full":
                    continue
                c = cs[i]
                if states[i] == "skip":
                    blk = np.zeros((128, 128), np.float32)
                else:
                    m = sub[c, :, j, :]
                    blk = (m.T != 0).astype(np.float32)
                hsh = hashlib.sha1(blk.tobytes()).hexdigest()
                if hsh not in uniq:
                    uniq[hsh] = len(uniq_src)
                    uniq_src.append(blk)
                mult_cols.append((i, uniq[hsh]))
            entries.append((j, cmin, mult_cols))
        plan.append(entries)
    mult_pack = (
        np.stack(uniq_src) if uniq_src else np.zeros((1, 128, 128), np.float32)
    )
    key = hashlib.sha1(
        repr([(qb, e) for qb, e in enumerate(plan)]).encode()
    ).hexdigest()
    return plan, mult_pack, key


def _build(mode, plan, n_mult):
    import concourse.mybir as mybir
    from concourse import bacc, tile
    from concourse import library_config

    f32 = mybir.dt.float32
    f16 = mybir.dt.float16
    bf16 = mybir.dt.bfloat16
    st_dt = bf16 if mode == "bf16" else f32

    AF = mybir.ActivationFunctionType
    AO = mybir.AluOpType

    nc = bacc.Bacc(
        "TRN2", target_bir_lowering=False, debug=False, num_devices=NCORES
    )

    io_dt = bf16 if mode == "bf16" else f32
    qT_d = nc.declare_dram_parameter("qT", [D, S], io_dt, isOutput=False).ap()
    kT_d = nc.declare_dram_parameter("kT", [D, S], io_dt, isOutput=False).ap()
    vT_d = nc.declare_dram_parameter("vT", [D, S], io_dt, isOutput=False).ap()
    wqT_d = nc.declare_dram_parameter("wqT", [D, GDIM], io_dt, isOutput=False).ap()
    wkT_d = nc.declare_dram_parameter("wkT", [D, GDIM], io_dt, isOutput=False).ap()
    wvT_d = nc.declare_dram_parameter("wvT", [D, GDIM], io_dt, isOutput=False).ap()
    woT_d = nc.declare_dram_parameter("woT", [GDIM, D], io_dt, isOutput=False).ap()
    bq_d = nc.declare_dram_parameter("bq2", [128, 2], f32, isOutput=False).ap()
    bk_d = nc.declare_dram_parameter("bk2", [128, 2], f32, isOutput=False).ap()
    mult_d = nc.declare_dram_parameter(
        "mult_pack", [n_mult, 128, 128], io_dt, isOutput=False
    ).ap()
    yT_d = nc.declare_dram_parameter("yT", [D, S], f16, isOutput=True).ap()

    with tile.TileContext(nc) as tc:
        with (
            tc.tile_pool(name="res", bufs=1) as res,
            tc.tile_pool(name="ot_pool", bufs=4) as ot_pool,
            tc.tile_pool(name="instream", bufs=25) as instream,
            tc.tile_pool(name="ptp", bufs=6) as ptp,
            tc.tile_pool(name="park", bufs=24) as park,
            tc.tile_pool(name="ystage", bufs=6) as ystage,
            tc.tile_pool(name="small", bufs=4) as small,
            tc.tile_pool(name="psum", bufs=2, space="PSUM") as psum,
        ):
            dma = nc.sync.dma_start
            dma_act = nc.scalar.dma_start

            nc.gpsimd.load_library(library_config.attn)

            wq_sb = res.tile([128, NDM, GDIM], st_dt, name="wq_sb")
            wk_sb = res.tile([128, NDM, GDIM], st_dt, name="wk_sb")
            wv_sb = res.tile([128, NDM, GDIM], st_dt, name="wv_sb")
            wo_sb = res.tile([128, 2, D], st_dt, name="wo_sb")
            bq_sb = res.tile([128, 2], f32, name="bq_sb")
            bk_sb = res.tile([128, 2], f32, name="bk_sb")
            mult_sb = res.tile([128, n_mult, 128], st_dt, name="mult_sb")

            QT_c = [res.tile([128, 2, 512], st_dt, name=f"QT{i}") for i in range(4)]
            KT_c = [res.tile([128, 2, 512], st_dt, name=f"KT{i}") for i in range(4)]
            V_c = [
                res.tile([128, 4, HPG, DK + 1], st_dt, name=f"V{i}")
                for i in range(4)
            ]

            def load_w(w_sb, w_d, dm, eng=None):
                (eng or dma)(
                    out=w_sb[:, dm, :],
                    in_=w_d[128 * dm : 128 * (dm + 1), :],
                )

            def stream_x2(src_d, p2, dm, eng=None):
                xt = instream.tile([128, 2 * QB], st_dt, name="xt", tag="xt")
                (eng or dma)(
                    out=xt,
                    in_=src_d[
                        128 * dm : 128 * (dm + 1),
                        2 * QB * p2 : 2 * QB * (p2 + 1),
                    ],
                )
                return xt

            def proj_qk2(p2, w_sb, b_sb, xts):
                dsts = (2 * p2, 2 * p2 + 1)
                out = []
                for ot in range(2):
                    ps = [
                        psum.tile([128, QB], f32, name="ps_p", tag="pp")
                        for _ in range(2)
                    ]
                    for dm in range(NDM):
                        w = w_sb[:, dm, 128 * ot : 128 * (ot + 1)]
                        for cl in range(2):
                            nc.tensor.matmul(
                                ps[cl],
                                lhsT=w,
                                rhs=xts[dm][:, QB * cl : QB * (cl + 1)],
                                start=(dm == 0),
                                stop=(dm == NDM - 1),
                            )
                    out.append(ps)
                return dsts, out

            def drain_qk2(dsts, out, dst_c, b_sb):
                for ot in range(2):
                    for cl in range(2):
                        nc.vector.tensor_scalar_add(
                            dst_c[dsts[cl]][:, ot, :],
                            out[ot][cl],
                            b_sb[:, ot : ot + 1],
                        )

            def proj_qk2_full(p2, w_sb, b_sb, dst_c, xts):
                dsts, out = proj_qk2(p2, w_sb, b_sb, xts)
                drain_qk2(dsts, out, dst_c, b_sb)

            def proj_v2(p2, xts):
                for c in range(2):
                    ci = 2 * p2 + c
                    for half in range(2):
                        ps_v = psum.tile(
                            [128, 2, GDIM], f32, name="ps_v", tag="pp"
                        )
                        for rl in range(2):
                            rt = 2 * half + rl
                            for dm in range(NDM):
                                nc.tensor.matmul(
                                    ps_v[:, rl, :],
                                    lhsT=xts[dm][
                                        :,
                                        QB * c + 128 * rt : QB * c + 128 * (rt + 1),
                                    ],
                                    rhs=wv_sb[:, dm, :],
                                    start=(dm == 0),
                                    stop=(dm == NDM - 1),
                                )
                        for rl in range(2):
                            nc.vector.tensor_copy(
                                out=V_c[ci][:, 2 * half + rl, :, 0:DK],
                                in_=ps_v[:, rl, :].rearrange(
                                    "p (h d) -> p h d", d=DK
                                ),
                            )

            def score_exp(qb, pr, entry, pool, tag=None):
                j, cmin, mult_cols = entry
                heads = (2 * pr, 2 * pr + 1)
                off = 128 * cmin
                jc, jl = j // 4, j % 4
                ps_s = psum.tile([128, 2, QB], f32, name="ps_s", tag="ss")
                for hh, h in enumerate(heads):
                    p0 = 64 * hh
                    ht = h // 2
                    nc.tensor.matmul(
                        ps_s[:, hh, off:QB],
                        lhsT=KT_c[jc][
                            p0 : p0 + 64, ht, 128 * jl : 128 * (jl + 1)
                        ],
                        rhs=QT_c[qb][p0 : p0 + 64, ht, off:QB],
                        start=True,
                        stop=True,
                    )
                kw = {"tag": tag} if tag else {}
                pt = pool.tile([128, 2, QB], st_dt, name="pt", **kw)
                nc.scalar.activation(
                    pt[:, :, off:QB],
                    ps_s[:, :, off:QB],
                    AF.Exp,
                    scale=0.125,
                )
                for cl, ui in mult_cols:
                    co = 128 * cl
                    for hh in range(2):
                        nc.vector.tensor_tensor(
                            out=pt[:, hh, co : co + 128],
                            in0=pt[:, hh, co : co + 128],
                            in1=mult_sb[:, ui, :],
                            op=AO.mult,
                        )
                return pt

            def attn_park(qb, pr, jmax):
                parked = {}
                for entry in plan[qb]:
                    if entry[0] > jmax:
                        break
                    parked[entry[0]] = score_exp(qb, pr, entry, park, tag="park")
                return parked

            def attn_pair(qb, pr, OT_sb, parked=None):
                entries = plan[qb]
                last_j = entries[-1][0]
                first_j = entries[0][0]
                heads = (2 * pr, 2 * pr + 1)
                po = {}
                for h in heads:
                    po[h] = psum.tile(
                        [DK + 1, QB], f32, name=f"po{h}", tag="po"
                    )
                for entry in entries:
                    j, cmin, mult_cols = entry
                    off = 128 * cmin
                    jc, jl = j // 4, j % 4
                    if parked is not None and j in parked:
                        pt = parked[j]
                    else:
                        pt = score_exp(qb, pr, entry, ptp)
                    for hh, h in enumerate(heads):
                        nc.tensor.matmul(
                            po[h][:, off:QB],
                            lhsT=V_c[jc][:, jl, h, :],
                            rhs=pt[:, hh, off:QB],
                            start=(j == first_j),
                            stop=(j == last_j),
                        )
                rl2 = small.tile([1, 2, QB], f32, name="rl2", tag="rl2")
                for hh, h in enumerate(heads):
                    nc.vector.reciprocal(
                        out=rl2[:, hh, :], in_=po[h][DK : DK + 1, :]
                    )
                rl_bc = small.tile([128, 2, QB], f32, name="rl_bc", tag="rl_bc")
                nc.gpsimd.partition_broadcast(
                    rl_bc[:, 0, :], rl2[:, 0, :], channels=128
                )
                nc.gpsimd.partition_broadcast(
                    rl_bc[:, 1, :], rl2[:, 1, :], channels=128
                )
                for hh, h in enumerate(heads):
                    p0 = 64 * hh
                    nc.vector.tensor_tensor(
                        out=OT_sb[p0 : p0 + 64, h // 2, :],
                        in0=po[h][0:DK, :],
                        in1=rl_bc[p0 : p0 + 64, hh, :],
                        op=AO.mult,
                    )

            def outproj(qb, OT_sb, on_act=False):
                for ot8 in range(8):
                    ps_y = psum.tile([128, QB], f32, name="ps_y", tag="pp")
                    for ct in range(2):
                        nc.tensor.matmul(
                            ps_y,
                            lhsT=wo_sb[:, ct, 128 * ot8 : 128 * (ot8 + 1)],
                            rhs=OT_sb[:, ct, :],
                            start=(ct == 0),
                            stop=(ct == 1),
                        )
                    ysb = ystage.tile([128, QB], f16, name="ysb")
                    if on_act:
                        nc.scalar.activation(ysb, ps_y, AF.Copy)
                    else:
                        nc.vector.tensor_copy(out=ysb, in_=ps_y)
                    dma(
                        out=yT_d[
                            128 * ot8 : 128 * (ot8 + 1),
                            QB * qb : QB * (qb + 1),
                        ],
                        in_=ysb,
                    )

            def outproj_tail(qbs, OTm):
                for ot8 in range(8):
                    ps = {
                        qb: psum.tile([128, QB], f32, name="ps_y", tag="pp")
                        for qb in qbs
                    }
                    for ct in range(2):
                        w = wo_sb[:, ct, 128 * ot8 : 128 * (ot8 + 1)]
                        for qb in qbs:
                            nc.tensor.matmul(
                                ps[qb],
                                lhsT=w,
                                rhs=OTm[qb][:, ct, :],
                                start=(ct == 0),
                                stop=(ct == 1),
                            )
                    for qi, qb in enumerate(qbs):
                        ysb = ystage.tile([128, QB], f16, name="ysb")
                        if (ot8 + qi) % 2 == 0:
                            nc.scalar.activation(ysb, ps[qb], AF.Copy)
                        else:
                            nc.vector.tensor_copy(out=ysb, in_=ps[qb])
                        dma(
                            out=yT_d[
                                128 * ot8 : 128 * (ot8 + 1),
                                QB * qb : QB * (qb + 1),
                            ],
                            in_=ysb,
                        )

            OTm = {}
            for qb in range(4):
                OTm[qb] = ot_pool.tile(
                    [128, 2, QB], st_dt, name=f"OT{qb}", tag="OT"
                )
            for i in range(4):
                nc.vector.memset(V_c[i][:, :, :, DK : DK + 1], 1.0)
            dma(out=bq_sb, in_=bq_d)
            dma(out=bk_sb, in_=bk_d)
            dma(out=mult_sb, in_=mult_d.rearrange("n p o -> p n o"))

            q01x, k01x, v01x = [], [], []
            for dm in range(NDM):
                load_w(wq_sb, wqT_d, dm)
                q01x.append(stream_x2(qT_d, 0, dm))
                load_w(wk_sb, wkT_d, dm, eng=dma_act)
                k01x.append(stream_x2(kT_d, 0, dm, eng=dma_act))
            proj_qk2_full(0, wq_sb, bq_sb, QT_c, q01x)
            for dm in range(NDM):
                load_w(wv_sb, wvT_d, dm, eng=dma_act)
                v01x.append(stream_x2(vT_d, 0, dm, eng=dma_act))
            proj_qk2_full(0, wk_sb, bk_sb, KT_c, k01x)
            proj_v2(0, v01x)
            for half in range(2):
                dma_act(
                    out=wo_sb[:, half, :],
                    in_=woT_d[128 * half : 128 * (half + 1), :],
                )

            attn_pair(0, 0, OTm[0])
            attn_pair(0, 1, OTm[0])
            q23x = [stream_x2(qT_d, 1, dm) for dm in range(NDM)]
            proj_qk2_full(1, wq_sb, bq_sb, QT_c, q23x)
            attn_pair(1, 0, OTm[1])
            k23x = [stream_x2(kT_d, 1, dm, eng=dma_act) for dm in range(NDM)]
            proj_qk2_full(1, wk_sb, bk_sb, KT_c, k23x)
            attn_pair(1, 1, OTm[1])
            v23x = [stream_x2(vT_d, 1, dm, eng=dma_act) for dm in range(NDM)]
            proj_v2(1, v23x)
            attn_pair(2, 0, OTm[2])
            outproj(0, OTm[0])
            park30 = attn_park(3, 0, jmax=11)
            attn_pair(2, 1, OTm[2])
            park31 = attn_park(3, 1, jmax=11)
            outproj(1, OTm[1])
            attn_pair(3, 0, OTm[3], parked=park30)
            attn_pair(3, 1, OTm[3], parked=park31)
            outproj_tail((2, 3), OTm)

    nc.compile()
    return nc


def _get_nc(mode, plan, n_mult, key):
    ck = (mode, key, n_mult)
    if ck not in _CACHE:
        _CACHE[ck] = _build(mode, plan, n_mult)
    return _CACHE[ck]


def _prep_inputs(q, k, v, wq, bq, wk, bk, wv, wo, mult_pack, mode):
    f32 = np.float32
    if mode == "bf16":
        import ml_dtypes

        io_np = ml_dtypes.bfloat16
    else:
        io_np = f32

    wqT = np.ascontiguousarray(np.asarray(wq, f32).T)
    wkT = np.ascontiguousarray(np.asarray(wk, f32).T)
    wvT = np.ascontiguousarray(np.asarray(wv, f32).T)
    woT = np.ascontiguousarray(np.asarray(wo, f32).T)

    in_maps = []
    for c in range(NCORES):
        b, g = c // GROUPS, c % GROUPS
        sl = slice(GDIM * g, GDIM * (g + 1))
        im = {
            "qT": np.ascontiguousarray(np.asarray(q[b], f32).T).astype(io_np),
            "kT": np.ascontiguousarray(np.asarray(k[b], f32).T).astype(io_np),
            "vT": np.ascontiguousarray(np.asarray(v[b], f32).T).astype(io_np),
            "wqT": np.ascontiguousarray(wqT[:, sl]).astype(io_np),
            "wkT": np.ascontiguousarray(wkT[:, sl]).astype(io_np),
            "wvT": np.ascontiguousarray(wvT[:, sl]).astype(io_np),
            "woT": np.ascontiguousarray(woT[sl, :]).astype(io_np),
            "bq2": np.ascontiguousarray(
                np.asarray(bq, f32)[sl].reshape(2, 128).T
            ),
            "bk2": np.ascontiguousarray(
                np.asarray(bk, f32)[sl].reshape(2, 128).T
            ),
            "mult_pack": mult_pack.astype(io_np),
        }
        in_maps.append(im)
    return in_maps


def _kernel_impl(q, k, v, mask, wq, bq, wk, bk, wv, bv, wo, bo, trace=False):
    from concourse.bass_utils import run_bass_kernel_spmd

    f32 = np.float32
    m2d = np.asarray(mask)[0, 0]
    plan, mult_pack, key = _make_plan(m2d)
    nc = _get_nc(MODE, plan, mult_pack.shape[0], key)
    in_maps = _prep_inputs(q, k, v, wq, bq, wk, bk, wv, wo, mult_pack, MODE)

    res = run_bass_kernel_spmd(nc, in_maps, list(range(NCORES)), trace=trace)

    bo_eff = (
        np.asarray(bo, np.float64)
        + np.asarray(bv, np.float64) @ np.asarray(wo, np.float64).T
    ).astype(f32)

    out = np.zeros((B, S, D), f32)
    for c in range(NCORES):
        out[c // GROUPS] += res.results[c]["yT"].astype(f32).T
    out += bo_eff
    return out, res


def kernel(q, k, v, mask, wq, bq, wk, bk, wv, bv, wo, bo):
    out, _ = _kernel_impl(q, k, v, mask, wq, bq, wk, bk, wv, bv, wo, bo)
    return out


# revision 6
# speedup vs baseline: 1.4552x; 1.4552x over previous
import os
import hashlib
import numpy as np

B, S, D, H, DK = 2, 2048, 1024, 16, 64
NCORES = 8
GROUPS = 4
HPG = 4
GDIM = HPG * DK
NEG = -1.0e9
QB = 512
NQB = S // QB
NKT = S // 128
NDM = D // 128

MODE = os.environ.get("BASS_MHA_MODE", "bf16")

_CACHE = {}


def _make_plan(m2d):
    sub = np.asarray(m2d).reshape(S // 128, 128, S // 128, 128)
    any_ = sub.any(axis=(1, 3))
    all_ = sub.all(axis=(1, 3))

    uniq = {}
    uniq_src = []
    plan = []
    for qb in range(NQB):
        entries = []
        cs = list(range(4 * qb, 4 * qb + 4))
        for j in range(NKT):
            states = []
            for c in cs:
                if not any_[c, j]:
                    states.append("skip")
                elif all_[c, j]:
                    states.append("full")
                else:
                    states.append("mixed")
            if all(s == "skip" for s in states):
                continue
            cmin = next(i for i, s in enumerate(states) if s != "skip")
            mult_cols = []
            for i in range(cmin, 4):
                if states[i] == "full":
                    continue
                c = cs[i]
                if states[i] == "skip":
                    blk = np.zeros((128, 128), np.float32)
                else:
                    m = sub[c, :, j, :]
                    blk = (m.T != 0).astype(np.float32)
                hsh = hashlib.sha1(blk.tobytes()).hexdigest()
                if hsh not in uniq:
                    uniq[hsh] = len(uniq_src)
                    uniq_src.append(blk)
                mult_cols.append((i, uniq[hsh]))
            entries.append((j, cmin, mult_cols))
        plan.append(entries)
    mult_pack = (
        np.stack(uniq_src) if uniq_src else np.zeros((1, 128, 128), np.float32)
    )
    key = hashlib.sha1(
        repr([(qb, e) for qb, e in enumerate(plan)]).encode()
    ).hexdigest()
    return plan, mult_pack, key


def _build(mode, plan, n_mult):
    import concourse.mybir as mybir
    from concourse import bacc, tile
    from concourse import library_config

    f32 = mybir.dt.float32
    f16 = mybir.dt.float16
    bf16 = mybir.dt.bfloat16
    st_dt = bf16 if mode == "bf16" else f32

    AF = mybir.ActivationFunctionType
    AO = mybir.AluOpType

    nc = bacc.Bacc(
        "TRN2", target_bir_lowering=False, debug=False, num_devices=NCORES
    )

    io_dt = bf16 if mode == "bf16" else f32
    qT_d = nc.declare_dram_parameter("qT", [D, S], io_dt, isOutput=False).ap()
    kT_d = nc.declare_dram_parameter("kT", [D, S], io_dt, isOutput=False).ap()
    vT_d = nc.declare_dram_parameter("vT", [D, S], io_dt, isOutput=False).ap()
    wqT_d = nc.declare_dram_parameter("wqT", [D, GDIM], io_dt, isOutput=False).ap()
    wkT_d = nc.declare_dram_parameter("wkT", [D, GDIM], io_dt, isOutput=False).ap()
    wvT_d = nc.declare_dram_parameter("wvT", [D, GDIM], io_dt, isOutput=False).ap()
    woT_d = nc.declare_dram_parameter("woT", [GDIM, D], io_dt, isOutput=False).ap()
    bq_d = nc.declare_dram_parameter("bq2", [128, 2], f32, isOutput=False).ap()
    bk_d = nc.declare_dram_parameter("bk2", [128, 2], f32, isOutput=False).ap()
    mult_d = nc.declare_dram_parameter(
        "mult_pack", [n_mult, 128, 128], io_dt, isOutput=False
    ).ap()
    yT_d = nc.declare_dram_parameter("yT", [D, S], f16, isOutput=True).ap()

    with tile.TileContext(nc) as tc:
        with (
            tc.tile_pool(name="res", bufs=1) as res,
            tc.tile_pool(name="ot_pool", bufs=4) as ot_pool,
            tc.tile_pool(name="instream", bufs=25) as instream,
            tc.tile_pool(name="ptp", bufs=6) as ptp,
            tc.tile_pool(name="park", bufs=24) as park,
            tc.tile_pool(name="ystage", bufs=6) as ystage,
            tc.tile_pool(name="small", bufs=2) as small,
            tc.tile_pool(name="psum", bufs=2, space="PSUM") as psum,
        ):
            dma = nc.sync.dma_start
            dma_act = nc.scalar.dma_start

            nc.gpsimd.load_library(library_config.attn)

            wq_sb = res.tile([128, NDM, GDIM], st_dt, name="wq_sb")
            wk_sb = res.tile([128, NDM, GDIM], st_dt, name="wk_sb")
            wv_sb = res.tile([128, NDM, GDIM], st_dt, name="wv_sb")
            wo_sb = res.tile([128, 2, D], st_dt, name="wo_sb")
            bq_sb = res.tile([128, 2], f32, name="bq_sb")
            bk_sb = res.tile([128, 2], f32, name="bk_sb")
            mult_sb = res.tile([128, n_mult, 128], st_dt, name="mult_sb")

            QT_c = [res.tile([128, 2, 512], st_dt, name=f"QT{i}") for i in range(4)]
            KT_c = [res.tile([128, 2, 512], st_dt, name=f"KT{i}") for i in range(4)]
            V_c = [
                res.tile([128, 4, HPG, DK + 1], st_dt, name=f"V{i}")
                for i in range(4)
            ]

            def load_w(w_sb, w_d, dm, eng=None):
                (eng or dma)(
                    out=w_sb[:, dm, :],
                    in_=w_d[128 * dm : 128 * (dm + 1), :],
                )

            def stream_x2(src_d, p2, dm, eng=None):
                xt = instream.tile([128, 2 * QB], st_dt, name="xt", tag="xt")
                (eng or dma)(
                    out=xt,
                    in_=src_d[
                        128 * dm : 128 * (dm + 1),
                        2 * QB * p2 : 2 * QB * (p2 + 1),
                    ],
                )
                return xt

            def proj_qk2(p2, w_sb, b_sb, xts):
                dsts = (2 * p2, 2 * p2 + 1)
                out = []
                for ot in range(2):
                    ps = [
                        psum.tile([128, QB], f32, name="ps_p", tag="pp")
                        for _ in range(2)
                    ]
                    for dm in range(NDM):
                        w = w_sb[:, dm, 128 * ot : 128 * (ot + 1)]
                        for cl in range(2):
                            nc.tensor.matmul(
                                ps[cl],
                                lhsT=w,
                                rhs=xts[dm][:, QB * cl : QB * (cl + 1)],
                                start=(dm == 0),
                                stop=(dm == NDM - 1),
                            )
                    out.append(ps)
                return dsts, out

            def drain_qk2(dsts, out, dst_c, b_sb):
                for ot in range(2):
                    for cl in range(2):
                        nc.vector.tensor_scalar_add(
                            dst_c[dsts[cl]][:, ot, :],
                            out[ot][cl],
                            b_sb[:, ot : ot + 1],
                        )

            def proj_qk2_full(p2, w_sb, b_sb, dst_c, xts):
                dsts, out = proj_qk2(p2, w_sb, b_sb, xts)
                drain_qk2(dsts, out, dst_c, b_sb)

            def proj_v2(p2, xts):
                for c in range(2):
                    ci = 2 * p2 + c
                    for half in range(2):
                        ps_v = psum.tile(
                            [128, 2, GDIM], f32, name="ps_v", tag="pp"
                        )
                        for rl in range(2):
                            rt = 2 * half + rl
                            for dm in range(NDM):
                                nc.tensor.matmul(
                                    ps_v[:, rl, :],
                                    lhsT=xts[dm][
                                        :,
                                        QB * c + 128 * rt : QB * c + 128 * (rt + 1),
                                    ],
                                    rhs=wv_sb[:, dm, :],
                                    start=(dm == 0),
                                    stop=(dm == NDM - 1),
                                )
                        for rl in range(2):
                            nc.vector.tensor_copy(
                                out=V_c[ci][:, 2 * half + rl, :, 0:DK],
                                in_=ps_v[:, rl, :].rearrange(
                                    "p (h d) -> p h d", d=DK
                                ),
                            )

            def score_exp(qb, pr, entry, pool, tag=None):
                j, cmin, mult_cols = entry
                heads = (2 * pr, 2 * pr + 1)
                off = 128 * cmin
                jc, jl = j // 4, j % 4
                ps_s = psum.tile([128, 2, QB], f32, name="ps_s", tag="ss")
                for hh, h in enumerate(heads):
                    p0 = 64 * hh
                    ht = h // 2
                    nc.tensor.matmul(
                        ps_s[:, hh, off:QB],
                        lhsT=KT_c[jc][
                            p0 : p0 + 64, ht, 128 * jl : 128 * (jl + 1)
                        ],
                        rhs=QT_c[qb][p0 : p0 + 64, ht, off:QB],
                        start=True,
                        stop=True,
                    )
                kw = {"tag": tag} if tag else {}
                pt = pool.tile([128, 2, QB], st_dt, name="pt", **kw)
                nc.scalar.activation(
                    pt[:, :, off:QB],
                    ps_s[:, :, off:QB],
                    AF.Exp,
                    scale=0.125,
                )
                for cl, ui in mult_cols:
                    co = 128 * cl
                    for hh in range(2):
                        nc.vector.tensor_tensor(
                            out=pt[:, hh, co : co + 128],
                            in0=pt[:, hh, co : co + 128],
                            in1=mult_sb[:, ui, :],
                            op=AO.mult,
                        )
                return pt

            def attn_park(qb, pr, jmax):
                parked = {}
                for entry in plan[qb]:
                    if entry[0] > jmax:
                        break
                    parked[entry[0]] = score_exp(qb, pr, entry, park, tag="park")
                return parked

            def attn_pair(qb, pr, OT_sb, parked=None):
                entries = plan[qb]
                last_j = entries[-1][0]
                first_j = entries[0][0]
                heads = (2 * pr, 2 * pr + 1)
                po = {}
                for h in heads:
                    po[h] = psum.tile(
                        [DK + 1, QB], f32, name=f"po{h}", tag="po"
                    )
                for entry in entries:
                    j, cmin, mult_cols = entry
                    off = 128 * cmin
                    jc, jl = j // 4, j % 4
                    if parked is not None and j in parked:
                        pt = parked[j]
                    else:
                        pt = score_exp(qb, pr, entry, ptp)
                    for hh, h in enumerate(heads):
                        nc.tensor.matmul(
                            po[h][:, off:QB],
                            lhsT=V_c[jc][:, jl, h, :],
                            rhs=pt[:, hh, off:QB],
                            start=(j == first_j),
                            stop=(j == last_j),
                        )
                l2 = small.tile([1, 2, QB], f32, name="l2", tag="l2")
                for hh, h in enumerate(heads):
                    nc.vector.tensor_copy(
                        out=l2[:, hh, :], in_=po[h][DK : DK + 1, :]
                    )
                rl2 = small.tile([1, 2, QB], f32, name="rl2", tag="rl2")
                nc.vector.reciprocal_approx_fast(out=rl2, in_=l2)
                rl_bc = small.tile([128, 2, QB], f32, name="rl_bc", tag="rl_bc")
                nc.gpsimd.partition_broadcast(
                    rl_bc[:, 0, :], rl2[:, 0, :], channels=128
                )
                nc.gpsimd.partition_broadcast(
                    rl_bc[:, 1, :], rl2[:, 1, :], channels=128
                )
                for hh, h in enumerate(heads):
                    p0 = 64 * hh
                    nc.vector.tensor_tensor(
                        out=OT_sb[p0 : p0 + 64, h // 2, :],
                        in0=po[h][0:DK, :],
                        in1=rl_bc[p0 : p0 + 64, hh, :],
                        op=AO.mult,
                    )

            def outproj(qb, OT_sb, on_act=False):
                for ot8 in range(8):
                    ps_y = psum.tile([128, QB], f32, name="ps_y", tag="pp")
                    for ct in range(2):
                        nc.tensor.matmul(
                            ps_y,
                            lhsT=wo_sb[:, ct, 128 * ot8 : 128 * (ot8 + 1)],
                            rhs=OT_sb[:, ct, :],
                            start=(ct == 0),
                            stop=(ct == 1),
                        )
                    ysb = ystage.tile([128, QB], f16, name="ysb")
                    if on_act:
                        nc.scalar.activation(ysb, ps_y, AF.Copy)
                    else:
                        nc.vector.tensor_copy(out=ysb, in_=ps_y)
                    dma(
                        out=yT_d[
                            128 * ot8 : 128 * (ot8 + 1),
                            QB * qb : QB * (qb + 1),
                        ],
                        in_=ysb,
                    )

            def outproj_tail(qbs, OTm):
                for ot8 in range(8):
                    ps = {
                        qb: psum.tile([128, QB], f32, name="ps_y", tag="pp")
                        for qb in qbs
                    }
                    for ct in range(2):
                        w = wo_sb[:, ct, 128 * ot8 : 128 * (ot8 + 1)]
                        for qb in qbs:
                            nc.tensor.matmul(
                                ps[qb],
                                lhsT=w,
                                rhs=OTm[qb][:, ct, :],
                                start=(ct == 0),
                                stop=(ct == 1),
                            )
                    for qi, qb in enumerate(qbs):
                        ysb = ystage.tile([128, QB], f16, name="ysb")
                        if (ot8 + qi) % 2 == 0:
                            nc.scalar.activation(ysb, ps[qb], AF.Copy)
                        else:
                            nc.vector.tensor_copy(out=ysb, in_=ps[qb])
                        dma(
                            out=yT_d[
                                128 * ot8 : 128 * (ot8 + 1),
                                QB * qb : QB * (qb + 1),
                            ],
                            in_=ysb,
                        )

            OTm = {}
            for qb in range(4):
                OTm[qb] = ot_pool.tile(
                    [128, 2, QB], st_dt, name=f"OT{qb}", tag="OT"
                )
            for i in range(4):
                nc.vector.memset(V_c[i][:, :, :, DK : DK + 1], 1.0)
            dma(out=bq_sb, in_=bq_d)
            dma(out=bk_sb, in_=bk_d)
            dma(out=mult_sb, in_=mult_d.rearrange("n p o -> p n o"))

            q01x, k01x, v01x = [], [], []
            for dm in range(NDM):
                load_w(wq_sb, wqT_d, dm)
                q01x.append(stream_x2(qT_d, 0, dm))
                load_w(wk_sb, wkT_d, dm, eng=dma_act)
                k01x.append(stream_x2(kT_d, 0, dm, eng=dma_act))
            proj_qk2_full(0, wq_sb, bq_sb, QT_c, q01x)
            for dm in range(NDM):
                load_w(wv_sb, wvT_d, dm, eng=dma_act)
                v01x.append(stream_x2(vT_d, 0, dm, eng=dma_act))
            proj_qk2_full(0, wk_sb, bk_sb, KT_c, k01x)
            proj_v2(0, v01x)
            for half in range(2):
                dma_act(
                    out=wo_sb[:, half, :],
                    in_=woT_d[128 * half : 128 * (half + 1), :],
                )

            attn_pair(0, 0, OTm[0])
            attn_pair(0, 1, OTm[0])
            q23x = [stream_x2(qT_d, 1, dm) for dm in range(NDM)]
            proj_qk2_full(1, wq_sb, bq_sb, QT_c, q23x)
            attn_pair(1, 0, OTm[1])
            k23x = [stream_x2(kT_d, 1, dm, eng=dma_act) for dm in range(NDM)]
            proj_qk2_full(1, wk_sb, bk_sb, KT_c, k23x)
            attn_pair(1, 1, OTm[1])
            v23x = [stream_x2(vT_d, 1, dm, eng=dma_act) for dm in range(NDM)]
            proj_v2(1, v23x)
            attn_pair(2, 0, OTm[2])
            outproj(0, OTm[0])
            park30 = attn_park(3, 0, jmax=11)
            attn_pair(2, 1, OTm[2])
            park31 = attn_park(3, 1, jmax=11)
            outproj(1, OTm[1])
            attn_pair(3, 0, OTm[3], parked=park30)
            attn_pair(3, 1, OTm[3], parked=park31)
            outproj_tail((2, 3), OTm)

    nc.compile()
    return nc


def _get_nc(mode, plan, n_mult, key):
    ck = (mode, key, n_mult)
    if ck not in _CACHE:
        _CACHE[ck] = _build(mode, plan, n_mult)
    return _CACHE[ck]


def _prep_inputs(q, k, v, wq, bq, wk, bk, wv, wo, mult_pack, mode):
    f32 = np.float32
    if mode == "bf16":
        import ml_dtypes

        io_np = ml_dtypes.bfloat16
    else:
        io_np = f32

    wqT = np.ascontiguousarray(np.asarray(wq, f32).T)
    wkT = np.ascontiguousarray(np.asarray(wk, f32).T)
    wvT = np.ascontiguousarray(np.asarray(wv, f32).T)
    woT = np.ascontiguousarray(np.asarray(wo, f32).T)

    in_maps = []
    for c in range(NCORES):
        b, g = c // GROUPS, c % GROUPS
        sl = slice(GDIM * g, GDIM * (g + 1))
        im = {
            "qT": np.ascontiguousarray(np.asarray(q[b], f32).T).astype(io_np),
            "kT": np.ascontiguousarray(np.asarray(k[b], f32).T).astype(io_np),
            "vT": np.ascontiguousarray(np.asarray(v[b], f32).T).astype(io_np),
            "wqT": np.ascontiguousarray(wqT[:, sl]).astype(io_np),
            "wkT": np.ascontiguousarray(wkT[:, sl]).astype(io_np),
            "wvT": np.ascontiguousarray(wvT[:, sl]).astype(io_np),
            "woT": np.ascontiguousarray(woT[sl, :]).astype(io_np),
            "bq2": np.ascontiguousarray(
                np.asarray(bq, f32)[sl].reshape(2, 128).T
            ),
            "bk2": np.ascontiguousarray(
                np.asarray(bk, f32)[sl].reshape(2, 128).T
            ),
            "mult_pack": mult_pack.astype(io_np),
        }
        in_maps.append(im)
    return in_maps


def _kernel_impl(q, k, v, mask, wq, bq, wk, bk, wv, bv, wo, bo, trace=False):
    from concourse.bass_utils import run_bass_kernel_spmd

    f32 = np.float32
    m2d = np.asarray(mask)[0, 0]
    plan, mult_pack, key = _make_plan(m2d)
    nc = _get_nc(MODE, plan, mult_pack.shape[0], key)
    in_maps = _prep_inputs(q, k, v, wq, bq, wk, bk, wv, wo, mult_pack, MODE)

    res = run_bass_kernel_spmd(nc, in_maps, list(range(NCORES)), trace=trace)

    bo_eff = (
        np.asarray(bo, np.float64)
        + np.asarray(bv, np.float64) @ np.asarray(wo, np.float64).T
    ).astype(f32)

    out = np.zeros((B, S, D), f32)
    for c in range(NCORES):
        out[c // GROUPS] += res.results[c]["yT"].astype(f32).T
    out += bo_eff
    return out, res


def kernel(q, k, v, mask, wq, bq, wk, bk, wv, bv, wo, bo):
    out, _ = _kernel_impl(q, k, v, mask, wq, bq, wk, bk, wv, bv, wo, bo)
    return out


# revision 9
# speedup vs baseline: 1.5399x; 1.0582x over previous
import os
import hashlib
import numpy as np

B, S, D, H, DK = 2, 2048, 1024, 16, 64
NCORES = 8
GROUPS = 4
HPG = 4
GDIM = HPG * DK
NEG = -1.0e9
QB = 512
NQB = S // QB
NKT = S // 128
NDM = D // 128

MODE = os.environ.get("BASS_MHA_MODE", "bf16")

_CACHE = {}


def _make_plan(m2d):
    sub = np.asarray(m2d).reshape(S // 128, 128, S // 128, 128)
    any_ = sub.any(axis=(1, 3))
    all_ = sub.all(axis=(1, 3))

    uniq = {}
    uniq_src = []
    plan = []
    for qb in range(NQB):
        entries = []
        cs = list(range(4 * qb, 4 * qb + 4))
        for j in range(NKT):
            states = []
            for c in cs:
                if not any_[c, j]:
                    states.append("skip")
                elif all_[c, j]:
                    states.append("full")
                else:
                    states.append("mixed")
            if all(s == "skip" for s in states):
                continue
            cmin = next(i for i, s in enumerate(states) if s != "skip")
            mult_cols = []
            for i in range(cmin, 4):
                if states[i] == "full":
                    continue
                c = cs[i]
                if states[i] == "skip":
                    blk = np.zeros((128, 128), np.float32)
                else:
                    m = sub[c, :, j, :]
                    blk = (m.T != 0).astype(np.float32)
                hsh = hashlib.sha1(blk.tobytes()).hexdigest()
                if hsh not in uniq:
                    uniq[hsh] = len(uniq_src)
                    uniq_src.append(blk)
                mult_cols.append((i, uniq[hsh]))
            entries.append((j, cmin, mult_cols))
        plan.append(entries)
    mult_pack = (
        np.stack(uniq_src) if uniq_src else np.zeros((1, 128, 128), np.float32)
    )
    key = hashlib.sha1(
        repr([(qb, e) for qb, e in enumerate(plan)]).encode()
    ).hexdigest()
    return plan, mult_pack, key


def _build(mode, plan, n_mult):
    import concourse.mybir as mybir
    from concourse import bacc, tile
    from concourse import library_config

    f32 = mybir.dt.float32
    f16 = mybir.dt.float16
    bf16 = mybir.dt.bfloat16
    st_dt = bf16 if mode == "bf16" else f32

    AF = mybir.ActivationFunctionType
    AO = mybir.AluOpType

    nc = bacc.Bacc(
        "TRN2", target_bir_lowering=False, debug=False, num_devices=NCORES
    )

    io_dt = bf16 if mode == "bf16" else f32
    qT_d = nc.declare_dram_parameter("qT", [D, S], io_dt, isOutput=False).ap()
    kT_d = nc.declare_dram_parameter("kT", [D, S], io_dt, isOutput=False).ap()
    vT_d = nc.declare_dram_parameter("vT", [D, S], io_dt, isOutput=False).ap()
    wqT_d = nc.declare_dram_parameter("wqT", [D, GDIM], io_dt, isOutput=False).ap()
    wkT_d = nc.declare_dram_parameter("wkT", [D, GDIM], io_dt, isOutput=False).ap()
    wvT_d = nc.declare_dram_parameter("wvT", [D, GDIM], io_dt, isOutput=False).ap()
    woT_d = nc.declare_dram_parameter("woT", [GDIM, D], io_dt, isOutput=False).ap()
    bq_d = nc.declare_dram_parameter("bq2", [128, 2], f32, isOutput=False).ap()
    bk_d = nc.declare_dram_parameter("bk2", [128, 2], f32, isOutput=False).ap()
    mult_d = nc.declare_dram_parameter(
        "mult_pack", [n_mult, 128, 128], io_dt, isOutput=False
    ).ap()
    yT_d = nc.declare_dram_parameter("yT", [D, S], f16, isOutput=True).ap()

    with tile.TileContext(nc) as tc:
        with (
            tc.tile_pool(name="res", bufs=1) as res,
            tc.tile_pool(name="ot_pool", bufs=4) as ot_pool,
            tc.tile_pool(name="instream", bufs=25) as instream,
            tc.tile_pool(name="ptp", bufs=5) as ptp,
            tc.tile_pool(name="park", bufs=32) as park,
            tc.tile_pool(name="ystage", bufs=6) as ystage,
            tc.tile_pool(name="small", bufs=2) as small,
            tc.tile_pool(name="psum", bufs=2, space="PSUM") as psum,
        ):
            dma = nc.sync.dma_start
            dma_act = nc.scalar.dma_start
            dma_gps = nc.gpsimd.dma_start

            nc.gpsimd.load_library(library_config.attn)

            wq_sb = res.tile([128, NDM, GDIM], st_dt, name="wq_sb")
            wk_sb = res.tile([128, NDM, GDIM], st_dt, name="wk_sb")
            wv_sb = res.tile([128, NDM, GDIM], st_dt, name="wv_sb")
            wo_sb = res.tile([128, 2, D], st_dt, name="wo_sb")
            bq_sb = res.tile([128, 2], f32, name="bq_sb")
            bk_sb = res.tile([128, 2], f32, name="bk_sb")
            mult_sb = res.tile([128, n_mult, 128], st_dt, name="mult_sb")

            QT_c = [res.tile([128, 2, 512], st_dt, name=f"QT{i}") for i in range(4)]
            KT_c = [res.tile([128, 2, 512], st_dt, name=f"KT{i}") for i in range(4)]
            V_c = [
                res.tile([128, 4, HPG, DK + 1], st_dt, name=f"V{i}")
                for i in range(4)
            ]

            def load_w(w_sb, w_d, dm, eng=None):
                (eng or dma)(
                    out=w_sb[:, dm, :],
                    in_=w_d[128 * dm : 128 * (dm + 1), :],
                )

            def stream_x2(src_d, p2, dm, eng=None):
                xt = instream.tile([128, 2 * QB], st_dt, name="xt", tag="xt")
                (eng or dma)(
                    out=xt,
                    in_=src_d[
                        128 * dm : 128 * (dm + 1),
                        2 * QB * p2 : 2 * QB * (p2 + 1),
                    ],
                )
                return xt

            def proj_qk(ci, w_sb, b_sb, dst, xts):
                cl = ci % 2
                for ot in range(2):
                    ps = psum.tile([128, QB], f32, name="ps_p", tag="pp")
                    for dm in range(NDM):
                        nc.tensor.matmul(
                            ps,
                            lhsT=w_sb[:, dm, 128 * ot : 128 * (ot + 1)],
                            rhs=xts[dm][:, QB * cl : QB * (cl + 1)],
                            start=(dm == 0),
                            stop=(dm == NDM - 1),
                        )
                    nc.vector.tensor_scalar_add(
                        dst[:, ot, :], ps, b_sb[:, ot : ot + 1]
                    )

            def proj_v(ci, xts):
                cl = ci % 2
                for half in range(2):
                    ps_v = psum.tile([128, 2, GDIM], f32, name="ps_v", tag="pp")
                    for rl in range(2):
                        rt = 2 * half + rl
                        for dm in range(NDM):
                            nc.tensor.matmul(
                                ps_v[:, rl, :],
                                lhsT=xts[dm][
                                    :,
                                    QB * cl + 128 * rt : QB * cl + 128 * (rt + 1),
                                ],
                                rhs=wv_sb[:, dm, :],
                                start=(dm == 0),
                                stop=(dm == NDM - 1),
                            )
                    for rl in range(2):
                        nc.vector.tensor_copy(
                            out=V_c[ci][:, 2 * half + rl, :, 0:DK],
                            in_=ps_v[:, rl, :].rearrange(
                                "p (h d) -> p h d", d=DK
                            ),
                        )

            def score_exp(qb, pr, entry, pool, tag=None):
                j, cmin, mult_cols = entry
                heads = (2 * pr, 2 * pr + 1)
                off = 128 * cmin
                jc, jl = j // 4, j % 4
                ps_s = psum.tile([128, 2, QB], f32, name="ps_s", tag="ss")
                for hh, h in enumerate(heads):
                    p0 = 64 * hh
                    ht = h // 2
                    nc.tensor.matmul(
                        ps_s[:, hh, off:QB],
                        lhsT=KT_c[jc][
                            p0 : p0 + 64, ht, 128 * jl : 128 * (jl + 1)
                        ],
                        rhs=QT_c[qb][p0 : p0 + 64, ht, off:QB],
                        start=True,
                        stop=True,
                    )
                kw = {"tag": tag} if tag else {}
                pt = pool.tile([128, 2, QB], st_dt, name="pt", **kw)
                nc.scalar.activation(
                    pt[:, :, off:QB],
                    ps_s[:, :, off:QB],
                    AF.Exp,
                    scale=0.125,
                )
                for cl, ui in mult_cols:
                    co = 128 * cl
                    for hh in range(2):
                        nc.vector.tensor_tensor(
                            out=pt[:, hh, co : co + 128],
                            in0=pt[:, hh, co : co + 128],
                            in1=mult_sb[:, ui, :],
                            op=AO.mult,
                        )
                return pt

            def attn_park(qb, pr, jmax):
                parked = {}
                for entry in plan[qb]:
                    if entry[0] > jmax:
                        break
                    parked[entry[0]] = score_exp(qb, pr, entry, park, tag="park")
                return parked

            def attn_pair(qb, pr, OT_sb, parked=None):
                entries = plan[qb]
                last_j = entries[-1][0]
                first_j = entries[0][0]
                heads = (2 * pr, 2 * pr + 1)
                po = {}
                for h in heads:
                    po[h] = psum.tile(
                        [DK + 1, QB], f32, name=f"po{h}", tag="po"
                    )
                for entry in entries:
                    j, cmin, mult_cols = entry
                    off = 128 * cmin
                    jc, jl = j // 4, j % 4
                    if parked is not None and j in parked:
                        pt = parked[j]
                    else:
                        pt = score_exp(qb, pr, entry, ptp)
                    for hh, h in enumerate(heads):
                        nc.tensor.matmul(
                            po[h][:, off:QB],
                            lhsT=V_c[jc][:, jl, h, :],
                            rhs=pt[:, hh, off:QB],
                            start=(j == first_j),
                            stop=(j == last_j),
                        )
                l2 = small.tile([1, 2, QB], f32, name="l2", tag="l2")
                for hh, h in enumerate(heads):
                    nc.vector.tensor_copy(
                        out=l2[:, hh, :], in_=po[h][DK : DK + 1, :]
                    )
                rl2 = small.tile([1, 2, QB], f32, name="rl2", tag="rl2")
                nc.vector.reciprocal_approx_fast(out=rl2, in_=l2)
                rl_bc = small.tile([128, 2, QB], f32, name="rl_bc", tag="rl_bc")
                nc.gpsimd.partition_broadcast(rl_bc, rl2, channels=128)
                for hh, h in enumerate(heads):
                    p0 = 64 * hh
                    nc.vector.tensor_tensor(
                        out=OT_sb[p0 : p0 + 64, h // 2, :],
                        in0=po[h][0:DK, :],
                        in1=rl_bc[p0 : p0 + 64, hh, :],
                        op=AO.mult,
                    )

            def outproj(qb, OT_sb, ots=range(8), on_act=False, yq=None):
                for u, ot8 in enumerate(ots):
                    ps_y = psum.tile([128, QB], f32, name="ps_y", tag="pp")
                    for ct in range(2):
                        nc.tensor.matmul(
                            ps_y,
                            lhsT=wo_sb[:, ct, 128 * ot8 : 128 * (ot8 + 1)],
                            rhs=OT_sb[:, ct, :],
                            start=(ct == 0),
                            stop=(ct == 1),
                        )
                    ysb = ystage.tile([128, QB], f16, name="ysb")
                    if on_act:
                        nc.scalar.activation(ysb, ps_y, AF.Copy)
                    else:
                        nc.vector.tensor_copy(out=ysb, in_=ps_y)
                    yeng = dma if yq is None else yq[u % len(yq)]
                    yeng(
                        out=yT_d[
                            128 * ot8 : 128 * (ot8 + 1),
                            QB * qb : QB * (qb + 1),
                        ],
                        in_=ysb,
                    )

            OTm = {}
            for qb in range(4):
                OTm[qb] = ot_pool.tile(
                    [128, 2, QB], st_dt, name=f"OT{qb}", tag="OT"
                )
            for i in range(4):
                nc.vector.memset(V_c[i][:, :, :, DK : DK + 1], 1.0)

            q01x, k01x, v01x = [], [], []
            for dm in range(NDM):
                load_w(wq_sb, wqT_d, dm)
                q01x.append(stream_x2(qT_d, 0, dm))
                load_w(wk_sb, wkT_d, dm, eng=dma_act)
                k01x.append(stream_x2(kT_d, 0, dm, eng=dma_act))
                load_w(wv_sb, wvT_d, dm, eng=dma_gps)
                v01x.append(stream_x2(vT_d, 0, dm, eng=dma_gps))
            dma(out=bq_sb, in_=bq_d)
            dma(out=bk_sb, in_=bk_d)
            dma(out=mult_sb, in_=mult_d.rearrange("n p o -> p n o"))
            for half in range(2):
                dma_act(
                    out=wo_sb[:, half, :],
                    in_=woT_d[128 * half : 128 * (half + 1), :],
                )

            proj_qk(0, wq_sb, bq_sb, QT_c[0], q01x)
            proj_qk(0, wk_sb, bk_sb, KT_c[0], k01x)
            proj_v(0, v01x)

            attn_pair(0, 0, OTm[0])
            proj_qk(1, wq_sb, bq_sb, QT_c[1], q01x)
            attn_pair(0, 1, OTm[0])
            proj_qk(1, wk_sb, bk_sb, KT_c[1], k01x)
            proj_v(1, v01x)
            attn_pair(1, 0, OTm[1])
            q23x = [stream_x2(qT_d, 1, dm) for dm in range(NDM)]
            proj_qk(2, wq_sb, bq_sb, QT_c[2], q23x)
            attn_pair(1, 1, OTm[1])
            k23x = [stream_x2(kT_d, 1, dm) for dm in range(NDM)]
            proj_qk(2, wk_sb, bk_sb, KT_c[2], k23x)
            v23x = [stream_x2(vT_d, 1, dm) for dm in range(NDM)]
            proj_v(2, v23x)
            attn_pair(2, 0, OTm[2])
            proj_qk(3, wq_sb, bq_sb, QT_c[3], q23x)
            outproj(0, OTm[0])
            attn_pair(2, 1, OTm[2])
            proj_qk(3, wk_sb, bk_sb, KT_c[3], k23x)
            park30 = attn_park(3, 0, jmax=15)
            proj_v(3, v23x)
            park31 = attn_park(3, 1, jmax=15)
            outproj(1, OTm[1])
            attn_pair(3, 0, OTm[3], parked=park30)
            outproj(2, OTm[2], ots=range(4))
            attn_pair(3, 1, OTm[3], parked=park31)
            outproj(2, OTm[2], ots=range(4, 8), on_act=True,
                    yq=[dma, dma_act])
            outproj(3, OTm[3], on_act=True, yq=[dma, dma_act])

    nc.compile()
    return nc


def _get_nc(mode, plan, n_mult, key):
    ck = (mode, key, n_mult)
    if ck not in _CACHE:
        _CACHE[ck] = _build(mode, plan, n_mult)
    return _CACHE[ck]


def _prep_inputs(q, k, v, wq, bq, wk, bk, wv, wo, mult_pack, mode):
    f32 = np.float32
    if mode == "bf16":
        import ml_dtypes

        io_np = ml_dtypes.bfloat16
    else:
        io_np = f32

    wqT = np.ascontiguousarray(np.asarray(wq, f32).T)
    wkT = np.ascontiguousarray(np.asarray(wk, f32).T)
    wvT = np.ascontiguousarray(np.asarray(wv, f32).T)
    woT = np.ascontiguousarray(np.asarray(wo, f32).T)

    in_maps = []
    for c in range(NCORES):
        b, g = c // GROUPS, c % GROUPS
        sl = slice(GDIM * g, GDIM * (g + 1))
        im = {
            "qT": np.ascontiguousarray(np.asarray(q[b], f32).T).astype(io_np),
            "kT": np.ascontiguousarray(np.asarray(k[b], f32).T).astype(io_np),
            "vT": np.ascontiguousarray(np.asarray(v[b], f32).T).astype(io_np),
            "wqT": np.ascontiguousarray(wqT[:, sl]).astype(io_np),
            "wkT": np.ascontiguousarray(wkT[:, sl]).astype(io_np),
            "wvT": np.ascontiguousarray(wvT[:, sl]).astype(io_np),
            "woT": np.ascontiguousarray(woT[sl, :]).astype(io_np),
            "bq2": np.ascontiguousarray(
                np.asarray(bq, f32)[sl].reshape(2, 128).T
            ),
            "bk2": np.ascontiguousarray(
                np.asarray(bk, f32)[sl].reshape(2, 128).T
            ),
            "mult_pack": mult_pack.astype(io_np),
        }
        in_maps.append(im)
    return in_maps


def _kernel_impl(q, k, v, mask, wq, bq, wk, bk, wv, bv, wo, bo, trace=False):
    from concourse.bass_utils import run_bass_kernel_spmd

    f32 = np.float32
    m2d = np.asarray(mask)[0, 0]
    plan, mult_pack, key = _make_plan(m2d)
    nc = _get_nc(MODE, plan, mult_pack.shape[0], key)
    in_maps = _prep_inputs(q, k, v, wq, bq, wk, bk, wv, wo, mult_pack, MODE)

    res = run_bass_kernel_spmd(nc, in_maps, list(range(NCORES)), trace=trace)

    bo_eff = (
        np.asarray(bo, np.float64)
        + np.asarray(bv, np.float64) @ np.asarray(wo, np.float64).T
    ).astype(f32)

    out = np.zeros((B, S, D), f32)
    for c in range(NCORES):
        out[c // GROUPS] += res.results[c]["yT"].astype(f32).T
    out += bo_eff
    return out, res


def kernel(q, k, v, mask, wq, bq, wk, bk, wv, bv, wo, bo):
    out, _ = _kernel_impl(q, k, v, mask, wq, bq, wk, bk, wv, bv, wo, bo)
    return out


# revision 10
# speedup vs baseline: 1.5768x; 1.0239x over previous
import os
import hashlib
import numpy as np

B, S, D, H, DK = 2, 2048, 1024, 16, 64
NCORES = 8
GROUPS = 4
HPG = 4
GDIM = HPG * DK
NEG = -1.0e9
QB = 512
NQB = S // QB
NKT = S // 128
NDM = D // 128

MODE = os.environ.get("BASS_MHA_MODE", "bf16")

_CACHE = {}


def _make_plan(m2d):
    sub = np.asarray(m2d).reshape(S // 128, 128, S // 128, 128)
    any_ = sub.any(axis=(1, 3))
    all_ = sub.all(axis=(1, 3))

    uniq = {}
    uniq_src = []
    plan = []
    for qb in range(NQB):
        entries = []
        cs = list(range(4 * qb, 4 * qb + 4))
        for j in range(NKT):
            states = []
            for c in cs:
                if not any_[c, j]:
                    states.append("skip")
                elif all_[c, j]:
                    states.append("full")
                else:
                    states.append("mixed")
            if all(s == "skip" for s in states):
                continue
            cmin = next(i for i, s in enumerate(states) if s != "skip")
            mult_cols = []
            for i in range(cmin, 4):
                if states[i] == "full":
                    continue
                c = cs[i]
                if states[i] == "skip":
                    blk = np.zeros((128, 128), np.float32)
                else:
                    m = sub[c, :, j, :]
                    blk = (m.T != 0).astype(np.float32)
                hsh = hashlib.sha1(blk.tobytes()).hexdigest()
                if hsh not in uniq:
                    uniq[hsh] = len(uniq_src)
                    uniq_src.append(blk)
                mult_cols.append((i, uniq[hsh]))
            entries.append((j, cmin, mult_cols))
        plan.append(entries)
    mult_pack = (
        np.stack(uniq_src) if uniq_src else np.zeros((1, 128, 128), np.float32)
    )
    key = hashlib.sha1(
        repr([(qb, e) for qb, e in enumerate(plan)]).encode()
    ).hexdigest()
    return plan, mult_pack, key


def _build(mode, plan, n_mult):
    import concourse.mybir as mybir
    from concourse import bacc, tile
    from concourse import library_config

    f32 = mybir.dt.float32
    f16 = mybir.dt.float16
    bf16 = mybir.dt.bfloat16
    st_dt = bf16 if mode == "bf16" else f32

    AF = mybir.ActivationFunctionType
    AO = mybir.AluOpType

    nc = bacc.Bacc(
        "TRN2", target_bir_lowering=False, debug=False, num_devices=NCORES
    )

    io_dt = bf16 if mode == "bf16" else f32
    qT_d = nc.declare_dram_parameter("qT", [D, S], io_dt, isOutput=False).ap()
    kT_d = nc.declare_dram_parameter("kT", [D, S], io_dt, isOutput=False).ap()
    vT_d = nc.declare_dram_parameter("vT", [D, S], io_dt, isOutput=False).ap()
    wqT_d = nc.declare_dram_parameter("wqT", [D, GDIM], io_dt, isOutput=False).ap()
    wkT_d = nc.declare_dram_parameter("wkT", [D, GDIM], io_dt, isOutput=False).ap()
    wvT_d = nc.declare_dram_parameter("wvT", [D, GDIM], io_dt, isOutput=False).ap()
    woT_d = nc.declare_dram_parameter("woT", [GDIM, D], io_dt, isOutput=False).ap()
    bq_d = nc.declare_dram_parameter("bq2", [128, 2], f32, isOutput=False).ap()
    bk_d = nc.declare_dram_parameter("bk2", [128, 2], f32, isOutput=False).ap()
    mult_d = nc.declare_dram_parameter(
        "mult_pack", [n_mult, 128, 128], io_dt, isOutput=False
    ).ap()
    yT_d = nc.declare_dram_parameter("yT", [D, S], f16, isOutput=True).ap()

    with tile.TileContext(nc) as tc:
        with (
            tc.tile_pool(name="res", bufs=1) as res,
            tc.tile_pool(name="ot_pool", bufs=4) as ot_pool,
            tc.tile_pool(name="instream", bufs=25) as instream,
            tc.tile_pool(name="ptp", bufs=5) as ptp,
            tc.tile_pool(name="park", bufs=32) as park,
            tc.tile_pool(name="ystage", bufs=6) as ystage,
            tc.tile_pool(name="small", bufs=2) as small,
            tc.tile_pool(name="psum", bufs=2, space="PSUM") as psum,
        ):
            dma = nc.sync.dma_start
            dma_act = nc.scalar.dma_start
            dma_gps = nc.gpsimd.dma_start

            nc.gpsimd.load_library(library_config.attn)

            wq_sb = res.tile([128, NDM, GDIM], st_dt, name="wq_sb")
            wk_sb = res.tile([128, NDM, GDIM], st_dt, name="wk_sb")
            wv_sb = res.tile([128, NDM, GDIM], st_dt, name="wv_sb")
            wo_sb = res.tile([128, 2, D], st_dt, name="wo_sb")
            bq_sb = res.tile([128, 2], f32, name="bq_sb")
            bk_sb = res.tile([128, 2], f32, name="bk_sb")
            mult_sb = res.tile([128, n_mult, 128], st_dt, name="mult_sb")

            QT_c = [res.tile([128, 2, 512], st_dt, name=f"QT{i}") for i in range(4)]
            KT_c = [res.tile([128, 2, 512], st_dt, name=f"KT{i}") for i in range(4)]
            V_c = [
                res.tile([128, 4, HPG, DK + 1], st_dt, name=f"V{i}")
                for i in range(4)
            ]

            def load_w(w_sb, w_d, dm, eng=None):
                (eng or dma)(
                    out=w_sb[:, dm, :],
                    in_=w_d[128 * dm : 128 * (dm + 1), :],
                )

            def stream_x2(src_d, p2, dm, eng=None):
                xt = instream.tile([128, 2 * QB], st_dt, name="xt", tag="xt")
                (eng or dma)(
                    out=xt,
                    in_=src_d[
                        128 * dm : 128 * (dm + 1),
                        2 * QB * p2 : 2 * QB * (p2 + 1),
                    ],
                )
                return xt

            def proj_qk(ci, w_sb, b_sb, dst, xts):
                cl = ci % 2
                for ot in range(2):
                    ps = psum.tile([128, QB], f32, name="ps_p", tag="pp")
                    for dm in range(NDM):
                        nc.tensor.matmul(
                            ps,
                            lhsT=w_sb[:, dm, 128 * ot : 128 * (ot + 1)],
                            rhs=xts[dm][:, QB * cl : QB * (cl + 1)],
                            start=(dm == 0),
                            stop=(dm == NDM - 1),
                        )
                    nc.vector.tensor_scalar_add(
                        dst[:, ot, :], ps, b_sb[:, ot : ot + 1]
                    )

            def proj_v(ci, xts):
                cl = ci % 2
                for half in range(2):
                    ps_v = psum.tile([128, 2, GDIM], f32, name="ps_v", tag="pp")
                    for rl in range(2):
                        rt = 2 * half + rl
                        for dm in range(NDM):
                            nc.tensor.matmul(
                                ps_v[:, rl, :],
                                lhsT=xts[dm][
                                    :,
                                    QB * cl + 128 * rt : QB * cl + 128 * (rt + 1),
                                ],
                                rhs=wv_sb[:, dm, :],
                                start=(dm == 0),
                                stop=(dm == NDM - 1),
                            )
                    for rl in range(2):
                        nc.vector.tensor_copy(
                            out=V_c[ci][:, 2 * half + rl, :, 0:DK],
                            in_=ps_v[:, rl, :].rearrange(
                                "p (h d) -> p h d", d=DK
                            ),
                        )

            def score_exp(qb, pr, entry, pool, tag=None):
                j, cmin, mult_cols = entry
                heads = (2 * pr, 2 * pr + 1)
                off = 128 * cmin
                jc, jl = j // 4, j % 4
                ps_s = psum.tile([128, 2, QB], f32, name="ps_s", tag="ss")
                for hh, h in enumerate(heads):
                    p0 = 64 * hh
                    ht = h // 2
                    nc.tensor.matmul(
                        ps_s[:, hh, off:QB],
                        lhsT=KT_c[jc][
                            p0 : p0 + 64, ht, 128 * jl : 128 * (jl + 1)
                        ],
                        rhs=QT_c[qb][p0 : p0 + 64, ht, off:QB],
                        start=True,
                        stop=True,
                    )
                kw = {"tag": tag} if tag else {}
                pt = pool.tile([128, 2, QB], st_dt, name="pt", **kw)
                nc.scalar.activation(
                    pt[:, :, off:QB],
                    ps_s[:, :, off:QB],
                    AF.Exp,
                    scale=0.125,
                )
                for cl, ui in mult_cols:
                    co = 128 * cl
                    for hh in range(2):
                        nc.vector.tensor_tensor(
                            out=pt[:, hh, co : co + 128],
                            in0=pt[:, hh, co : co + 128],
                            in1=mult_sb[:, ui, :],
                            op=AO.mult,
                        )
                return pt

            def attn_park(qb, pr, jmax):
                parked = {}
                for entry in plan[qb]:
                    if entry[0] > jmax:
                        break
                    parked[entry[0]] = score_exp(qb, pr, entry, park, tag="park")
                return parked

            def attn_pair(qb, pr, OT_sb, parked=None):
                entries = plan[qb]
                last_j = entries[-1][0]
                first_j = entries[0][0]
                heads = (2 * pr, 2 * pr + 1)
                po = {}
                for h in heads:
                    po[h] = psum.tile(
                        [DK + 1, QB], f32, name=f"po{h}", tag="po"
                    )
                for entry in entries:
                    j, cmin, mult_cols = entry
                    off = 128 * cmin
                    jc, jl = j // 4, j % 4
                    if parked is not None and j in parked:
                        pt = parked[j]
                    else:
                        pt = score_exp(qb, pr, entry, ptp)
                    for hh, h in enumerate(heads):
                        nc.tensor.matmul(
                            po[h][:, off:QB],
                            lhsT=V_c[jc][:, jl, h, :],
                            rhs=pt[:, hh, off:QB],
                            start=(j == first_j),
                            stop=(j == last_j),
                        )
                l2 = small.tile([1, 2, QB], f32, name="l2", tag="l2")
                for hh, h in enumerate(heads):
                    nc.vector.tensor_copy(
                        out=l2[:, hh, :], in_=po[h][DK : DK + 1, :]
                    )
                rl2 = small.tile([1, 2, QB], f32, name="rl2", tag="rl2")
                nc.vector.reciprocal_approx_fast(out=rl2, in_=l2)
                rl_bc = small.tile([128, 2, QB], f32, name="rl_bc", tag="rl_bc")
                nc.gpsimd.partition_broadcast(rl_bc, rl2, channels=128)
                for hh, h in enumerate(heads):
                    p0 = 64 * hh
                    nc.vector.tensor_tensor(
                        out=OT_sb[p0 : p0 + 64, h // 2, :],
                        in0=po[h][0:DK, :],
                        in1=rl_bc[p0 : p0 + 64, hh, :],
                        op=AO.mult,
                    )

            def outproj(qb, OT_sb, ots=range(8), on_act=False, yq=None):
                for u, ot8 in enumerate(ots):
                    ps_y = psum.tile([128, QB], f32, name="ps_y", tag="pp")
                    for ct in range(2):
                        nc.tensor.matmul(
                            ps_y,
                            lhsT=wo_sb[:, ct, 128 * ot8 : 128 * (ot8 + 1)],
                            rhs=OT_sb[:, ct, :],
                            start=(ct == 0),
                            stop=(ct == 1),
                        )
                    ysb = ystage.tile([128, QB], f16, name="ysb")
                    if on_act:
                        nc.scalar.activation(ysb, ps_y, AF.Copy)
                    else:
                        nc.vector.tensor_copy(out=ysb, in_=ps_y)
                    yeng = dma if yq is None else yq[u % len(yq)]
                    yeng(
                        out=yT_d[
                            128 * ot8 : 128 * (ot8 + 1),
                            QB * qb : QB * (qb + 1),
                        ],
                        in_=ysb,
                    )

            OTm = {}
            for qb in range(4):
                OTm[qb] = ot_pool.tile(
                    [128, 2, QB], st_dt, name=f"OT{qb}", tag="OT"
                )
            for i in range(4):
                nc.vector.memset(V_c[i][:, :, :, DK : DK + 1], 1.0)

            q01x, k01x, v01x = [], [], []
            for dm in range(NDM):
                load_w(wv_sb, wvT_d, dm, eng=dma_gps)
            for dm in range(NDM):
                load_w(wq_sb, wqT_d, dm)
                q01x.append(stream_x2(qT_d, 0, dm))
                load_w(wk_sb, wkT_d, dm, eng=dma_act)
                k01x.append(stream_x2(kT_d, 0, dm, eng=dma_act))
            for dm in range(NDM):
                v01x.append(stream_x2(vT_d, 0, dm))
            for half in range(2):
                dma_act(
                    out=wo_sb[:, half, :],
                    in_=woT_d[128 * half : 128 * (half + 1), :],
                )
            dma_act(out=bq_sb, in_=bq_d)
            dma_act(out=bk_sb, in_=bk_d)
            dma_act(out=mult_sb, in_=mult_d.rearrange("n p o -> p n o"))

            proj_qk(0, wq_sb, bq_sb, QT_c[0], q01x)
            proj_qk(0, wk_sb, bk_sb, KT_c[0], k01x)
            proj_qk(1, wq_sb, bq_sb, QT_c[1], q01x)
            proj_qk(1, wk_sb, bk_sb, KT_c[1], k01x)
            proj_v(0, v01x)

            attn_pair(0, 0, OTm[0])
            attn_pair(0, 1, OTm[0])
            proj_v(1, v01x)
            attn_pair(1, 0, OTm[1])
            q23x = [stream_x2(qT_d, 1, dm) for dm in range(NDM)]
            proj_qk(2, wq_sb, bq_sb, QT_c[2], q23x)
            attn_pair(1, 1, OTm[1])
            k23x = [stream_x2(kT_d, 1, dm) for dm in range(NDM)]
            proj_qk(2, wk_sb, bk_sb, KT_c[2], k23x)
            v23x = [stream_x2(vT_d, 1, dm) for dm in range(NDM)]
            proj_v(2, v23x)
            attn_pair(2, 0, OTm[2])
            proj_qk(3, wq_sb, bq_sb, QT_c[3], q23x)
            outproj(0, OTm[0])
            attn_pair(2, 1, OTm[2])
            proj_qk(3, wk_sb, bk_sb, KT_c[3], k23x)
            park30 = attn_park(3, 0, jmax=15)
            proj_v(3, v23x)
            park31 = attn_park(3, 1, jmax=15)
            outproj(1, OTm[1])
            attn_pair(3, 0, OTm[3], parked=park30)
            outproj(2, OTm[2], ots=range(4))
            attn_pair(3, 1, OTm[3], parked=park31)
            outproj(2, OTm[2], ots=range(4, 8), on_act=True,
                    yq=[dma, dma_act])
            outproj(3, OTm[3], on_act=True, yq=[dma, dma_act])

    nc.compile()
    return nc


def _get_nc(mode, plan, n_mult, key):
    ck = (mode, key, n_mult)
    if ck not in _CACHE:
        _CACHE[ck] = _build(mode, plan, n_mult)
    return _CACHE[ck]


def _prep_inputs(q, k, v, wq, bq, wk, bk, wv, wo, mult_pack, mode):
    f32 = np.float32
    if mode == "bf16":
        import ml_dtypes

        io_np = ml_dtypes.bfloat16
    else:
        io_np = f32

    wqT = np.ascontiguousarray(np.asarray(wq, f32).T)
    wkT = np.ascontiguousarray(np.asarray(wk, f32).T)
    wvT = np.ascontiguousarray(np.asarray(wv, f32).T)
    woT = np.ascontiguousarray(np.asarray(wo, f32).T)

    in_maps = []
    for c in range(NCORES):
        b, g = c // GROUPS, c % GROUPS
        sl = slice(GDIM * g, GDIM * (g + 1))
        im = {
            "qT": np.ascontiguousarray(np.asarray(q[b], f32).T).astype(io_np),
            "kT": np.ascontiguousarray(np.asarray(k[b], f32).T).astype(io_np),
            "vT": np.ascontiguousarray(np.asarray(v[b], f32).T).astype(io_np),
            "wqT": np.ascontiguousarray(wqT[:, sl]).astype(io_np),
            "wkT": np.ascontiguousarray(wkT[:, sl]).astype(io_np),
            "wvT": np.ascontiguousarray(wvT[:, sl]).astype(io_np),
            "woT": np.ascontiguousarray(woT[sl, :]).astype(io_np),
            "bq2": np.ascontiguousarray(
                np.asarray(bq, f32)[sl].reshape(2, 128).T
            ),
            "bk2": np.ascontiguousarray(
                np.asarray(bk, f32)[sl].reshape(2, 128).T
            ),
            "mult_pack": mult_pack.astype(io_np),
        }
        in_maps.append(im)
    return in_maps


def _kernel_impl(q, k, v, mask, wq, bq, wk, bk, wv, bv, wo, bo, trace=False):
    from concourse.bass_utils import run_bass_kernel_spmd

    f32 = np.float32
    m2d = np.asarray(mask)[0, 0]
    plan, mult_pack, key = _make_plan(m2d)
    nc = _get_nc(MODE, plan, mult_pack.shape[0], key)
    in_maps = _prep_inputs(q, k, v, wq, bq, wk, bk, wv, wo, mult_pack, MODE)

    res = run_bass_kernel_spmd(nc, in_maps, list(range(NCORES)), trace=trace)

    bo_eff = (
        np.asarray(bo, np.float64)
        + np.asarray(bv, np.float64) @ np.asarray(wo, np.float64).T
    ).astype(f32)

    out = np.zeros((B, S, D), f32)
    for c in range(NCORES):
        out[c // GROUPS] += res.results[c]["yT"].astype(f32).T
    out += bo_eff
    return out, res


def kernel(q, k, v, mask, wq, bq, wk, bk, wv, bv, wo, bo):
    out, _ = _kernel_impl(q, k, v, mask, wq, bq, wk, bk, wv, bv, wo, bo)
    return out
